# revision 20
# baseline (speedup 1.0000x reference)
"""Multi-head self-attention (B=4, S=2048, E=1024, H=16, causal) on 8 TRN2
NeuronCores, tensor-parallel over heads (2 heads/core).

Per-core pipeline (matmuls bf16, fp32 PSUM accumulation):
  1. QKV projection from a host-transposed query qT [E, T]:
       Q^T,K^T [128(2h*64d), T] via lhsT=w^T chunks; V [t,130] natural layout
       with ones columns at 0 and 65 (PV sum rows). K gets NO bias (softmax is
       invariant to per-query score shifts, so only Q's bias matters); Q bias
       added on DVE (tensor_scalar) so ScalarE runs pure exp; b_v folded into
       the output bias on the host (b_out_eff = b_out + w_out @ b_v).
       qt tiles are prefetched one t-block ahead of their matmuls.
  2. Causal attention in S^T layout (keys on partitions, queries on free dim),
     both heads per (b, j) unit: the two score matmuls use row groups
     0-63/64-127 (tile_position auto-derived) and distinct PSUM banks, so they
     run concurrently. P = exp(S/8) in one strided ACT call over both heads;
     diagonal k-blocks are trimmed to the causal q-range (scores, exp, mask,
     PV all shrink). PV via lhsT=V_aug (ones col -> row 0 = softmax sums),
     normalization deferred past the A2A; attnU row 0 carries the sums.
     Each pass emits a flat, software-pipelined kb stream (scores of kb i+1
     before PV of kb i, across unit boundaries) with proj / phase-3 fillers
     spliced in at unit closes.
  3. Two half-AllToAlls (even/odd 512-token q-blocks) with per-batch staging
     DMAs. Phase 3 of half 0 is interleaved into pass 2 once the half-0 A2A
     is safely complete; per-token reciprocal (DVE, overlapped with the af
     prefetch) + PE-broadcast normalization, output projection, bf16
     out^T [E, 1024] per core with per-chunk DMA; host concatenates.
"""
import sys

if "/opt/trn_rl_repo" not in sys.path:
    sys.path.insert(0, "/opt/trn_rl_repo")

import numpy as np
import ml_dtypes

BF16 = ml_dtypes.bfloat16

B, S, E, H, D = 4, 2048, 1024, 16, 64
T = B * S  # 8192
N_CORES = 8
HPC = H // N_CORES  # 2 heads per core
TL = T // N_CORES  # 1024 tokens per core for the output shard
NTB = T // 512  # 16 projection t-blocks
SCALE = 1.0 / np.sqrt(D)

_CACHE = {}


def build_kernel():
    import concourse.mybir as mybir
    import concourse.tile as tile
    from concourse import bacc
    from concourse.bass import ds, ts

    F32 = mybir.dt.float32
    BF = mybir.dt.bfloat16
    AF = mybir.ActivationFunctionType
    ALU = mybir.AluOpType

    nc = bacc.Bacc("TRN2", target_bir_lowering=False, debug=False,
                   num_devices=N_CORES)

    qT_d = nc.dram_tensor("qT", [E, T], BF, kind="ExternalInput")
    wqk_d = nc.dram_tensor("wqk", [E, 256], BF, kind="ExternalInput")
    wv_d = nc.dram_tensor("wv", [E, 128], BF, kind="ExternalInput")
    bq_d = nc.dram_tensor("bq", [128, 1], F32, kind="ExternalInput")
    wout_d = nc.dram_tensor("wout", [E, E], BF, kind="ExternalInput")
    bout_d = nc.dram_tensor("bout", [128, 8], F32, kind="ExternalInput")
    masks_d = nc.dram_tensor("masks", [128, 8, 512], BF, kind="ExternalInput")
    sel_d = nc.dram_tensor("sel", [16, 8, 128], BF, kind="ExternalInput")
    outT_d = nc.dram_tensor("outT", [E, TL], BF, kind="ExternalOutput")

    with tile.TileContext(nc) as tc:
        with (
            tc.tile_pool(name="consts", bufs=1) as cpool,
            tc.tile_pool(name="dram", bufs=1, space="DRAM") as dram,
            tc.tile_pool(name="spair", bufs=2, space="PSUM") as ps_pair,
            tc.tile_pool(name="att", bufs=2, space="PSUM") as ps_att,
            tc.tile_pool(name="psv", bufs=2, space="PSUM") as ps_v,
            tc.tile_pool(name="persist", bufs=1) as ppool,
            tc.tile_pool(name="qt", bufs=3) as qtpool,
            tc.tile_pool(name="pex", bufs=4) as ppex,
            tc.tile_pool(name="ph3", bufs=2) as p3,
        ):
            # ---- phase-1 constants first (qt loads must not queue behind wout)
            wqk_sb = cpool.tile([128, 8, 256], BF)
            nc.sync.dma_start(wqk_sb[:], wqk_d.ap().rearrange("(c p) f -> p c f", p=128))
            wv_sb = cpool.tile([128, 8, 128], BF)
            nc.sync.dma_start(wv_sb[:], wv_d.ap().rearrange("(c p) f -> p c f", p=128))
            bq_sb = cpool.tile([128, 1], F32)
            nc.sync.dma_start(bq_sb[:], bq_d.ap())
            masks_sb = cpool.tile([128, 8, 512], BF)
            nc.sync.dma_start(masks_sb[:], masks_d.ap())
            # later-phase constants declared now, loaded after proj starts
            wout_sb = cpool.tile([128, 8, 1024], BF)
            bout_sb = cpool.tile([128, 8], F32)
            sel_sb = cpool.tile([16, 8, 128], BF)

            q_sb = ppool.tile([128, T], BF, tag="q_sb")
            k_sb = ppool.tile([128, T], BF, tag="k_sb")
            # V per 128-token block: [ones | h0 d0-63 | ones | h1 d0-63]
            v_sb = ppool.tile([128, 64, 130], BF, tag="v_sb")
            # row 0: softmax sums; rows 1-64: unnormalized attn^T per head
            attnU = ppool.tile([65, 2, T], BF, tag="attnU")

            nc.vector.memset(v_sb[:, :, 0:1], 1.0)
            nc.vector.memset(v_sb[:, :, 65:66], 1.0)

            # ---- phase 1: QKV projection over 512-token blocks
            qT_r = qT_d.ap().rearrange("(c p) t -> p c t", p=128)
            qt_tiles = {}

            def qt_load(tb):
                qt = qtpool.tile([128, 8, 512], BF, name="qt")
                nc.sync.dma_start(qt[:], qT_r[:, :, ts(tb, 512)])
                qt_tiles[tb] = qt

            def proj_mms(tb):
                qt = qt_tiles.pop(tb)
                ps = ps_pair.tile([128, 1024], F32, tag="sp", name="ps")
                for c in range(8):
                    nc.tensor.matmul(ps[:, 0:512], wqk_sb[:, c, 0:128],
                                     qt[:, c, :], start=(c == 0), stop=(c == 7))
                for c in range(8):
                    nc.tensor.matmul(ps[:, 512:1024], wqk_sb[:, c, 128:256],
                                     qt[:, c, :], start=(c == 0), stop=(c == 7))
                nc.vector.tensor_scalar_add(q_sb[:, ts(tb, 512)], ps[:, 0:512],
                                            bq_sb[:, 0:1])
                nc.vector.tensor_copy(k_sb[:, ts(tb, 512)], ps[:, 512:1024])
                for sub in range(4):
                    t128 = tb * 4 + sub
                    psv = ps_v.tile([128, 128], F32, tag="psv", name="psv")
                    for c in range(8):
                        nc.tensor.matmul(psv[:], qt[:, c, ds(sub * 128, 128)],
                                         wv_sb[:, c, :], start=(c == 0), stop=(c == 7))
                    nc.vector.tensor_copy(
                        v_sb[:, t128, :].rearrange("p (h c) -> p h c", h=2)[:, :, ds(1, 64)],
                        psv[:].rearrange("p (h c) -> p h c", h=2))

            # ---- phase 2: software-pipelined kb stream, both heads per kb
            att_tiles = {}

            def scores_exp(b, j, kb):
                q0 = b * S + j * 512
                m = kb - 4 * j  # >= 0: diagonal block index
                off = 128 * m if m > 0 else 0
                w = 512 - off
                k0 = b * S + kb * 128
                sp = ps_pair.tile([128, 1024], F32, tag="sp", name="sp")
                nc.tensor.matmul(sp[:, ds(off, w)],
                                 k_sb[ds(0, 64), ds(k0, 128)],
                                 q_sb[ds(0, 64), ds(q0 + off, w)],
                                 start=True, stop=True)
                nc.tensor.matmul(sp[:, ds(512 + off, w)],
                                 k_sb[ds(64, 64), ds(k0, 128)],
                                 q_sb[ds(64, 64), ds(q0 + off, w)],
                                 start=True, stop=True)
                p = ppex.tile([128, 1024], BF, name="p")
                pv = p[:].rearrange("k (h q) -> k h q", h=2)[:, :, ds(off, w)]
                spv = sp[:].rearrange("k (h q) -> k h q", h=2)[:, :, ds(off, w)]
                nc.scalar.activation(pv, spv, AF.Exp, scale=SCALE)
                if m >= 0:
                    nc.vector.tensor_tensor(
                        pv, pv, masks_sb[:, ds(2 * m, 2), ds(off, w)],
                        op=ALU.mult)
                return p, off, w

            def pv_mm(b, j, kb, p, off, w):
                nkb = 4 * j + 4
                if kb == 0:
                    att_tiles[(b, j)] = [
                        ps_att.tile([65, 512], F32, tag="att", name=f"att{b}_{j}_{h}")
                        for h in range(2)]
                att = att_tiles[(b, j)]
                t128 = b * 16 + kb
                for h in range(2):
                    dst = att[h][:, ds(off, w)] if off else att[h][:]
                    nc.tensor.matmul(dst, v_sb[:, t128, ds(65 * h, 65)],
                                     p[:, ds(512 * h + off, w)],
                                     start=(kb == 0), stop=(kb == nkb - 1))
                if kb == nkb - 1:
                    q0 = b * S + j * 512
                    att = att_tiles.pop((b, j))
                    for h in range(2):
                        nc.vector.tensor_copy(attnU[:, h, ds(q0, 512)], att[h][:])

            def emit_pass(units, pre, post):
                """units: ordered (b, j) list. pre[ui]: callables emitted
                before unit ui's first scores (proj mms the unit depends on).
                post[ui]: callables emitted after unit ui's last PV+epilogue
                (staging, phase-3 work). The kb stream is software-pipelined:
                scores of kb i+1 are emitted before the PV of kb i, across
                unit boundaries."""
                stream = [(ui, b, j, kb)
                          for ui, (b, j) in enumerate(units)
                          for kb in range(4 * j + 4)]
                pend = None
                for ui, b, j, kb in stream:
                    if kb == 0:
                        for f in pre.get(ui, ()):
                            f()
                    cur = scores_exp(b, j, kb)
                    if pend is not None:
                        pui, pb, pj, pkb, pp = pend
                        pv_mm(pb, pj, pkb, *pp)
                        if pkb == 4 * pj + 3:
                            for f in post.get(pui, ()):
                                f()
                    pend = (ui, b, j, kb, cur)
                ui, b, j, kb, pp = pend
                pv_mm(b, j, kb, *pp)
                for f in post.get(ui, ()):
                    f()

            # ---- phase 3 helpers
            a2a_in = [dram.tile([N_CORES, 130, 512], BF, tag=f"a2a_in{i}",
                                name=f"a2a_in{i}") for i in range(2)]
            a2a_out = [dram.tile([N_CORES, 130, 512], BF, tag=f"a2a_out{i}",
                                 name=f"a2a_out{i}") for i in range(2)]

            def stage_half(half, b):
                """Stage batch b's attnU slice for the half-A2A (dest cores
                2b, 2b+1) as soon as its units complete."""
                src = attnU[:].rearrange("p h (c t) -> p h c t", c=N_CORES)
                sl = ds(half * 512, 512)
                cb = ds(2 * b, 2)
                nc.sync.dma_start(
                    a2a_in[half][cb, 0:64, :].rearrange("c p t -> p c t"),
                    src[1:65, 0, cb, sl])
                nc.sync.dma_start(
                    a2a_in[half][cb, 64:128, :].rearrange("c p t -> p c t"),
                    src[1:65, 1, cb, sl])
                for h in range(2):
                    nc.sync.dma_start(
                        a2a_in[half][cb, ds(128 + h, 1), :],
                        src[0:1, h, cb, sl])

            def a2a(half):
                nc.gpsimd.collective_compute(
                    "AllToAll", ALU.bypass,
                    replica_groups=[list(range(N_CORES))],
                    ins=[a2a_in[half][:].opt()], outs=[a2a_out[half][:].opt()])

            def phase3_prefetch(half):
                rsrc = p3.tile([16, 512], BF, tag="rsrc", name=f"rsrc{half}")
                nc.sync.dma_start(rsrc[:], a2a_out[half][:, 128:130, :])
                af = p3.tile([128, 8, 512], BF, tag="af", name=f"af{half}")
                for c in range(8):
                    nc.sync.dma_start(af[:, c, :], a2a_out[half][c, 0:128, :])
                return af, rsrc

            def phase3_recip(half, rsrc):
                rbf = p3.tile([16, 512], BF, tag="rbf", name=f"rbf{half}")
                rf32 = p3.tile([16, 512], F32, tag="rf32", name=f"rf32_{half}")
                nc.vector.reciprocal(rf32[:], rsrc[:])
                nc.vector.tensor_copy(rbf[:], rf32[:])
                return rbf

            def phase3_norm(half, af, rbf):
                """Broadcast 1/sums to 128 partitions via sel-matmul, multiply
                into af."""
                for c in range(8):
                    rb = ps_att.tile([128, 512], F32, tag="att", name=f"rb{half}_{c}")
                    nc.tensor.matmul(rb[:], sel_sb[:, c, :], rbf[:],
                                     start=True, stop=True)
                    nc.vector.tensor_tensor(af[:, c, :], af[:, c, :],
                                            rb[:], op=ALU.mult)

            def phase3_out(half, af, ms):
                """Output projection chunks m in ms, with per-chunk DMA."""
                outT_r = outT_d.ap().rearrange("(m p) t -> p m t", p=128)
                for m in ms:
                    po = ps_v.tile([128, 512], F32, tag="psv", name=f"po{half}_{m}")
                    for c in range(8):
                        nc.tensor.matmul(po[:], wout_sb[:, c, ds(m * 128, 128)],
                                         af[:, c, :], start=(c == 0), stop=(c == 7))
                    osb = p3.tile([128, 512], BF, tag="osb", name=f"osb{half}_{m}")
                    nc.scalar.activation(osb[:], po[:], AF.Identity,
                                         bias=bout_sb[:, ds(m, 1)])
                    nc.sync.dma_start(outT_r[:, m, ts(half, 512)], osb[:])

            def load_phase3_consts():
                nc.sync.dma_start(
                    wout_sb[:], wout_d.ap().rearrange("(c p) e -> p c e", p=128))
                nc.sync.dma_start(bout_sb[:], bout_d.ap())
                nc.sync.dma_start(sel_sb[:], sel_d.ap())

            # ---- emission schedule
            # pass 1: even q-blocks (A2A half 0). proj t-blocks paired to the
            # first unit that needs them, qt DMAs >= 1 unit ahead of the mms
            # (qt pool bufs=3); per-b staging so the A2A fires right after the
            # last unit. tb 4b+3 is deferred to pass 2 (only U(b,3) needs it).
            qt_load(0)
            qt_load(1)
            qt_load(2)
            units1 = [(b, j) for b in range(B) for j in (0, 2)]
            pre1 = {
                0: [lambda: proj_mms(0)],
                1: [lambda: proj_mms(1), lambda: proj_mms(2)],
                2: [lambda: proj_mms(4)],
                3: [lambda: proj_mms(5), lambda: proj_mms(6)],
                4: [lambda: proj_mms(8)],
                5: [lambda: proj_mms(9), lambda: proj_mms(10)],
                6: [lambda: proj_mms(12)],
                7: [lambda: proj_mms(13), lambda: proj_mms(14)],
            }
            post1 = {
                0: [load_phase3_consts, lambda: qt_load(4)],
                1: [lambda: stage_half(0, 0), lambda: qt_load(5),
                    lambda: qt_load(6)],
                2: [lambda: qt_load(8)],
                3: [lambda: stage_half(0, 1), lambda: qt_load(9),
                    lambda: qt_load(10)],
                4: [lambda: qt_load(12)],
                5: [lambda: stage_half(0, 2), lambda: qt_load(13),
                    lambda: qt_load(14)],
                6: [lambda: qt_load(3)],
                7: [lambda: stage_half(0, 3)],
            }
            emit_pass(units1, pre1, post1)
            a2a(0)
            af0, rsrc0 = phase3_prefetch(0)
            # pass 2: odd q-blocks (A2A half 1); phase 3 of half 0 interleaved
            # once the half-0 A2A is safely complete.
            units2 = [(b, j) for b in range(B) for j in (1, 3)]
            rbf0 = []
            pre2 = {
                0: [lambda: proj_mms(3)],
                3: [lambda: proj_mms(7)],
                5: [lambda: proj_mms(11)],
                7: [lambda: proj_mms(15)],
            }
            post2 = {
                0: [lambda: qt_load(7)],
                1: [lambda: stage_half(1, 0), lambda: qt_load(11)],
                3: [lambda: stage_half(1, 1), lambda: qt_load(15)],
                5: [lambda: stage_half(1, 2),
                    lambda: rbf0.append(phase3_recip(0, rsrc0))],
                6: [lambda: phase3_norm(0, af0, rbf0[0]),
                    lambda: phase3_out(0, af0, [0, 1, 2, 3])],
                7: [lambda: stage_half(1, 3)],
            }
            emit_pass(units2, pre2, post2)
            a2a(1)
            af1, rsrc1 = phase3_prefetch(1)
            phase3_out(0, af0, [4, 5, 6, 7])  # fills the half-1 A2A window
            rbf1 = phase3_recip(1, rsrc1)
            phase3_norm(1, af1, rbf1)
            phase3_out(1, af1, list(range(8)))

    nc.compile()
    return nc


def prep_inputs(query, w_in, b_in, w_out, b_out):
    """Shard + lay out host-side. Returns in_maps for the 8 cores."""
    query = np.asarray(query, dtype=np.float32)
    w_in = np.asarray(w_in, dtype=np.float32)
    b_in = np.asarray(b_in, dtype=np.float32)
    w_out = np.asarray(w_out, dtype=np.float32)
    b_out = np.asarray(b_out, dtype=np.float32)

    qT = np.ascontiguousarray(query.reshape(T, E).T).astype(BF16)
    woutT = np.ascontiguousarray(w_out.T).astype(BF16)
    b_v = b_in[2 * E:3 * E]
    bout_eff = (b_out + w_out @ b_v).reshape(8, 128).T.copy()  # [128, 8]

    # causal masks for the 4 diagonal 128x512 blocks, duplicated per head:
    # masks[p, 2m+h, q] = p <= q - 128m
    qidx = np.arange(512)[None, :]
    pidx = np.arange(128)[:, None]
    masks = np.stack([(pidx <= qidx - 128 * m) for m in range(4)
                      for _ in range(2)], axis=1)
    masks = masks.astype(BF16)  # [128, 8, 512]

    sel = np.zeros((16, 8, 128), dtype=BF16)
    for c in range(8):
        sel[2 * c, c, 0:64] = 1.0
        sel[2 * c + 1, c, 64:128] = 1.0

    in_maps = []
    for c in range(N_CORES):
        r = slice(128 * c, 128 * c + 128)
        wqk = np.concatenate([w_in[:E][r].T, w_in[E:2 * E][r].T], axis=1)
        wv = w_in[2 * E:3 * E][r].T
        bq = b_in[:E][r][:, None]
        in_maps.append({
            "qT": qT,
            "wqk": np.ascontiguousarray(wqk).astype(BF16),
            "wv": np.ascontiguousarray(wv).astype(BF16),
            "bq": np.ascontiguousarray(bq),
            "wout": woutT,
            "bout": np.ascontiguousarray(bout_eff),
            "masks": masks,
            "sel": sel,
        })
    return in_maps


def run_on_hw(in_maps, trace=False, **kw):
    from concourse.bass_utils import run_bass_kernel_spmd

    if "nc" not in _CACHE:
        _CACHE["nc"] = build_kernel()
    return run_bass_kernel_spmd(_CACHE["nc"], in_maps, list(range(N_CORES)),
                                trace=trace, **kw)


def kernel(query, w_in, b_in, w_out, b_out):
    in_maps = prep_inputs(query, w_in, b_in, w_out, b_out)
    res = run_on_hw(in_maps)
    parts = [np.asarray(res.results[c]["outT"], dtype=np.float32).T
             for c in range(N_CORES)]  # [TL, E] each
    out = np.concatenate(parts, axis=0).reshape(B, S, E)
    return out.astype(np.float32)
